# revision 11
# baseline (speedup 1.0000x reference)
"""DeepSAE Trainium2 kernel: 8-core feature-sharded, exact global top-k.

Pipeline per core (feature shard of 3072 of 24576 features):
  per token-block n: enc1 (replicated) -> mid0T_n; enc2 all feature tiles -> pre1T HBM
  after block 0: sample (all features x 16 tokens) -> allgather -> replicated
    bisection -> bracket (t_lo, t_hi)
  capture: top-8 per (feature x 64-token) of bracket-shifted values (Sterbenz-exact)
  condense -> allgather candidates + allreduce count-above-hi
  final replicated bisection -> exact global k-th largest t*
  dec2: stream pre1T, mask (x>=t*)*x, partial mid2 -> ReduceScatter (token slices)
  dec1: relu(mid2+b)@W_dec1+b -> reconT token slice + SSE partials
Host: concat recon slices, sum SSE -> loss.
"""

import numpy as np

D_MODEL, D_MID, D_FEAT = 768, 1536, 24576
B, S = 4, 512
T = B * S                      # 2048 tokens
NC = 8
DF = D_FEAT // NC              # 3072 features per core
TSL = T // NC                  # 256-token output slice per core
K_TOTAL = 64 * T               # 131072


def build_tile_kernel(tc, ins, outs):
    import concourse.mybir as mybir

    nc = tc.nc
    f32 = mybir.dt.float32
    AF = mybir.ActivationFunctionType
    OP = mybir.AluOpType

    xT = ins["xT"]          # [768, 2048]
    xTs = ins["xTs"]        # [768, 256]   this core's token slice (for SSE)
    W1 = ins["W1"]          # [768, 1536]
    b1 = ins["b1"]          # [1536]
    W2 = ins["W2"]          # [1536, 3072] shard
    b2 = ins["b2"]          # [3072] shard
    Wd2 = ins["Wd2"]        # [3072, 1536] shard
    bd2 = ins["bd2"]        # [1536]
    Wd1 = ins["Wd1"]        # [1536, 768]
    bd1 = ins["bd1"]        # [768]
    reconT = outs["reconT"]  # [768, 256]
    sse = outs["sse"]        # [128, 6]
    dbg = outs["dbg"]        # [1, 8]

    RG = [list(range(NC))]

    import contextlib
    est = contextlib.ExitStack()
    with est:
        const = est.enter_context(tc.tile_pool(name="const", bufs=1))
        dram = est.enter_context(tc.tile_pool(name="dram", bufs=1, space="DRAM"))

        ones_col = const.tile([128, 1], f32)
        nc.vector.memset(ones_col[:], 1.0)
        ones_row = const.tile([1, 128], f32)
        nc.vector.memset(ones_row[:], 1.0)
        t_lo = const.tile([128, 1], f32)
        t_hi = const.tile([128, 1], f32)
        t_star = const.tile([128, 1], f32)
        rank_b = const.tile([128, 1], f32)
        neg_hi_sc = const.tile([128, 1], f32)

        b2_sb = const.tile([128, 24], f32)
        nc.sync.dma_start(b2_sb[:], b2.rearrange("(o p) -> p o", p=128))
        bd2_sb = const.tile([128, 12], f32)
        nc.sync.dma_start(bd2_sb[:], bd2.rearrange("(o p) -> p o", p=128))
        bd1_sb = const.tile([128, 6], f32)
        nc.sync.dma_start(bd1_sb[:], bd1.rearrange("(o p) -> p o", p=128))

        pre1T = dram.tile([DF, T], f32)
        ch_g = dram.tile([1], f32)

        # ================= phases 1-3: encode + select =================
        with tc.tile_pool(name="sel", bufs=1) as sel:
            cap_sb = sel.tile([128, 24 * 32 * 8], f32)   # capture slots (+ scratch)
            cnthi_st = sel.tile([128, 96], f32)
            sample_sb = sel.tile([128, 3072], f32)
            comp = sel.tile([128, 144], f32)
            cand = sel.tile([128, 8 * 144], f32)
            st = sel.tile([128, 8], f32)
            smp_l = sel.tile([128, 384], f32)
            ch_all = sel.tile([1, 1], f32)
            mid_t = sel.tile([128, 2], f32)

            with tc.tile_pool(name="enc", bufs=1) as enc, \
                 tc.tile_pool(name="encs", bufs=2) as encs, \
                 tc.tile_pool(name="w2s", bufs=2) as w2s, \
                 tc.tile_pool(name="stg", bufs=4) as stg, \
                 tc.tile_pool(name="eps", bufs=3, space="PSUM") as eps, \
                 tc.tile_pool(name="sps", bufs=1, space="PSUM") as sps, \
                 tc.tile_pool(name="cin", bufs=2) as cin:

                W1_sb = enc.tile([128, 6, D_MID], f32)
                nc.sync.dma_start(
                    W1_sb[:], W1.rearrange("(ko ki) m -> ki ko m", ki=128))
                b1_sb = enc.tile([128, 12], f32)
                nc.sync.dma_start(b1_sb[:], b1.rearrange("(o p) -> p o", p=128))

                def global_count(data_ap, mid_ap, scr_ap):
                    """bcast count(data > mid) over all partitions -> [128,1]"""
                    cnt = stg.tile([128, 1], f32, tag="cnt")
                    nc.vector.tensor_scalar(scr_ap, data_ap, mid_ap, None,
                                            op0=OP.is_gt, op1=OP.add,
                                            accum_out=cnt[:])
                    pa = sps.tile([1, 1], f32, tag="pa")
                    nc.tensor.matmul(pa[:], cnt[:], ones_col[:],
                                     start=True, stop=True)
                    tot = stg.tile([1, 1], f32, tag="tot")
                    nc.scalar.copy(tot[:], pa[:])
                    pb = sps.tile([128, 1], f32, tag="pb")
                    nc.tensor.matmul(pb[:], ones_row[:], tot[:],
                                     start=True, stop=True)
                    cb = stg.tile([128, 1], f32, tag="cb")
                    nc.scalar.copy(cb[:], pb[:])
                    return cb

                for n in range(4):
                    xn = encs.tile([128, 6, 512], f32, tag="xn")
                    nc.sync.dma_start(
                        xn[:], xT[:, n * 512:(n + 1) * 512]
                        .rearrange("(ko ki) t -> ki ko t", ki=128))
                    mid0n = encs.tile([128, 12, 512], f32, tag="mid0n")
                    for m12 in range(12):
                        pse = eps.tile([128, 512], f32, tag="eps1")
                        for kc in range(6):
                            nc.tensor.matmul(
                                pse[:], W1_sb[:, kc, m12 * 128:(m12 + 1) * 128],
                                xn[:, kc, :], start=(kc == 0), stop=(kc == 5))
                        nc.scalar.activation(mid0n[:, m12, :], pse[:], AF.Relu,
                                             bias=b1_sb[:, m12:m12 + 1])
                    for m in range(24):
                        w2t = w2s.tile([128, 12, 128], f32, tag="w2t")
                        nc.sync.dma_start(
                            w2t[:], W2[:, m * 128:(m + 1) * 128]
                            .rearrange("(ko ki) f -> ki ko f", ki=128))
                        pst = eps.tile([128, 512], f32, tag="eps2")
                        for kc in range(12):
                            nc.tensor.matmul(pst[:], w2t[:, kc, :],
                                             mid0n[:, kc, :],
                                             start=(kc == 0), stop=(kc == 11))
                        so = stg.tile([128, 512], f32, tag="stg")
                        nc.scalar.activation(so[:], pst[:], AF.Relu,
                                             bias=b2_sb[:, m:m + 1])
                        nc.sync.dma_start(
                            pre1T[m * 128:(m + 1) * 128,
                                  n * 512:(n + 1) * 512], so[:])

                    if n == 0:
                        # sample: all features x tokens 0:512:32 -> allgather
                        for mo in range(24):
                            nc.sync.dma_start(
                                smp_l[:, mo * 16:(mo + 1) * 16],
                                pre1T[mo * 128:(mo + 1) * 128, 0:512:32])
                        smp_bnc = dram.tile([128 * 384], f32)
                        smp_g = dram.tile([NC * 128 * 384], f32)
                        nc.sync.dma_start(
                            smp_bnc[:].rearrange("(p t) -> p t", p=128),
                            smp_l[:])
                        nc.gpsimd.collective_compute(
                            "AllGather", OP.bypass, replica_groups=RG,
                            ins=[smp_bnc[:]], outs=[smp_g[:]])
                        nc.sync.dma_start(
                            sample_sb[:].rearrange("p (c j) -> p c j", c=NC),
                            smp_g[:].rearrange("(c p j) -> p c j", c=NC, p=128))

                        # replicated sample bisection (2 searches interleaved)
                        nc.vector.memset(st[:, 0:1], 0.0)
                        nc.vector.memset(st[:, 1:2], 128.0)
                        nc.vector.memset(st[:, 2:3], 0.0)
                        nc.vector.memset(st[:, 3:4], 128.0)
                        r_full = K_TOTAL // 128
                        targets = [float(r_full - 144) + 0.5,
                                   float(r_full + 144) + 0.5]
                        for _it in range(20):
                            for s2 in range(2):
                                lo_c = st[:, 2 * s2:2 * s2 + 1]
                                hi_c = st[:, 2 * s2 + 1:2 * s2 + 2]
                                mid_c = mid_t[:, s2:s2 + 1]
                                nc.vector.tensor_add(mid_c, lo_c, hi_c)
                                nc.vector.tensor_scalar_mul(mid_c, mid_c, 0.5)
                                cb = global_count(sample_sb[:], mid_c,
                                                  cap_sb[:, 0:3072])
                                selm = stg.tile([128, 1], mybir.dt.uint32, tag="selm")
                                nc.vector.tensor_scalar(
                                    selm[:], cb[:], targets[s2], None,
                                    op0=OP.is_ge)
                                selmi = stg.tile([128, 1], mybir.dt.uint32, tag="selmi")
                                nc.vector.tensor_scalar(
                                    selmi[:], cb[:], targets[s2], None,
                                    op0=OP.is_lt)
                                nc.vector.copy_predicated(lo_c, selm[:], mid_c)
                                nc.vector.copy_predicated(hi_c, selmi[:], mid_c)
                        nc.vector.tensor_copy(t_hi[:], st[:, 1:2])
                        nc.vector.tensor_copy(t_lo[:], st[:, 2:3])
                        nc.vector.tensor_scalar_mul(neg_hi_sc[:], t_hi[:],
                                                    -1.0e9)

                # ---- capture pass over all 96 tiles (re-read from HBM)
                for m in range(24):
                    for n in range(4):
                        t96 = m * 4 + n
                        xa = cin.tile([128, 512], f32, tag="xa")
                        nc.sync.dma_start(
                            xa[:], pre1T[m * 128:(m + 1) * 128,
                                         n * 512:(n + 1) * 512])
                        at = cin.tile([128, 512], f32, tag="at")
                        nc.scalar.activation(at[:], xa[:], AF.Relu,
                                             bias=neg_hi_sc[:], scale=1.0e9)
                        jk = cin.tile([128, 512], f32, tag="jk")
                        nc.scalar.activation(
                            jk[:], at[:], AF.Sign,
                            accum_out=cnthi_st[:, t96:t96 + 1])
                        yt = cin.tile([128, 512], f32, tag="yt")
                        nc.vector.scalar_tensor_tensor(
                            yt[:], xa[:], t_lo[:], at[:],
                            op0=OP.subtract, op1=OP.subtract)
                        for e in range(8):
                            col = ((n * 8 + e) * 24 + m) * 8
                            nc.vector.max(cap_sb[:, col:col + 8],
                                          yt[:, e * 64:(e + 1) * 64])

            # ========== phase 3: condense + gather + final search ==========
            with tc.tile_pool(name="cnd", bufs=2) as cnd, \
                 tc.tile_pool(name="fps", bufs=1, space="PSUM") as fps, \
                 tc.tile_pool(name="fst", bufs=2) as fst:

                cond = cnd.tile([128, 512], f32, tag="cond", bufs=1)
                for s in range(32):
                    seg = cap_sb[:, s * 192:(s + 1) * 192]
                    nc.vector.max(cond[:, s * 16:s * 16 + 8], seg)
                    zap = cnd.tile([128, 192], f32, tag="zap")
                    nc.vector.match_replace(zap[:], cond[:, s * 16:s * 16 + 8],
                                            seg, -1.0e9)
                    nc.vector.max(cond[:, s * 16 + 8:s * 16 + 16], zap[:])
                cur = cond
                for r in range(18):
                    nc.vector.max(comp[:, r * 8:r * 8 + 8], cur[:])
                    if r < 17:
                        nxt = cnd.tile([128, 512], f32, tag="cur")
                        nc.vector.match_replace(nxt[:], comp[:, r * 8:r * 8 + 8],
                                                cur[:], -1.0e9)
                        cur = nxt

                # count_hi -> allreduce ; candidates -> allgather
                chl = fst.tile([128, 1], f32, tag="chl")
                nc.vector.tensor_reduce(chl[:], cnthi_st[:],
                                        axis=mybir.AxisListType.X,
                                        op=mybir.AluOpType.add)
                pa = fps.tile([1, 1], f32, tag="pa")
                nc.tensor.matmul(pa[:], chl[:], ones_col[:],
                                 start=True, stop=True)
                ch_sb = fst.tile([1, 1], f32, tag="chsb")
                nc.scalar.copy(ch_sb[:], pa[:])
                ch_bnc = dram.tile([1], f32)
                nc.sync.dma_start(ch_bnc[:].rearrange("(a b) -> a b", a=1), ch_sb[:])
                nc.gpsimd.collective_compute(
                    "AllReduce", OP.add, replica_groups=RG,
                    ins=[ch_bnc[:]], outs=[ch_g[:]])
                nc.sync.dma_start(ch_all[:],
                                  ch_g[:].rearrange("(a b) -> a b", a=1))

                cd_bnc = dram.tile([128 * 144], f32)
                cd_g = dram.tile([NC * 128 * 144], f32)
                nc.sync.dma_start(cd_bnc[:].rearrange("(p j) -> p j", p=128), comp[:])
                nc.gpsimd.collective_compute(
                    "AllGather", OP.bypass, replica_groups=RG,
                    ins=[cd_bnc[:]], outs=[cd_g[:]])
                nc.sync.dma_start(
                    cand[:].rearrange("p (c j) -> p c j", c=NC),
                    cd_g[:].rearrange("(c p j) -> p c j", c=NC, p=128))
                nc.vector.tensor_scalar(cand[:], cand[:], t_lo[:], None,
                                        op0=OP.add)

                rk = fst.tile([1, 1], f32, tag="rk")
                nc.vector.tensor_scalar(rk[:], ch_all[:], float(K_TOTAL), None,
                                        op0=OP.subtract)   # cnt - K
                nc.vector.tensor_scalar_mul(rk[:], rk[:], -1.0)
                pb = fps.tile([128, 1], f32, tag="pb")
                nc.tensor.matmul(pb[:], ones_row[:], rk[:], start=True, stop=True)
                nc.scalar.copy(rank_b[:], pb[:])

                flo = fst.tile([128, 1], f32, tag="flo")
                fhi = fst.tile([128, 1], f32, tag="fhi")
                nc.vector.tensor_copy(flo[:], t_lo[:])
                nc.vector.tensor_scalar(fhi[:], t_hi[:], 1.0e-3, None, op0=OP.add)
                fmid = fst.tile([128, 1], f32, tag="fmid")
                for _it in range(30):
                    nc.vector.tensor_add(fmid[:], flo[:], fhi[:])
                    nc.vector.tensor_scalar_mul(fmid[:], fmid[:], 0.5)
                    cnt = fst.tile([128, 1], f32, tag="fcnt")
                    nc.vector.tensor_scalar(cap_sb[:, 0:1152], cand[:], fmid[:],
                                            None, op0=OP.is_gt, op1=OP.add,
                                            accum_out=cnt[:])
                    pc = fps.tile([1, 1], f32, tag="pc")
                    nc.tensor.matmul(pc[:], cnt[:], ones_col[:],
                                     start=True, stop=True)
                    tt = fst.tile([1, 1], f32, tag="ftot")
                    nc.scalar.copy(tt[:], pc[:])
                    pd = fps.tile([128, 1], f32, tag="pd")
                    nc.tensor.matmul(pd[:], ones_row[:], tt[:],
                                     start=True, stop=True)
                    cb2 = fst.tile([128, 1], f32, tag="fcb")
                    nc.scalar.copy(cb2[:], pd[:])
                    selm2 = fst.tile([128, 1], mybir.dt.uint32, tag="fsel")
                    nc.vector.tensor_tensor(selm2[:], cb2[:], rank_b[:],
                                            op=OP.is_ge)
                    selm2i = fst.tile([128, 1], mybir.dt.uint32, tag="fseli")
                    nc.vector.tensor_tensor(selm2i[:], cb2[:], rank_b[:],
                                            op=OP.is_lt)
                    nc.vector.copy_predicated(flo[:], selm2[:], fmid[:])
                    nc.vector.copy_predicated(fhi[:], selm2i[:], fmid[:])
                nc.vector.tensor_copy(t_star[:], fhi[:])

                nc.sync.dma_start(dbg[0:1, 0:1], t_star[0:1, :])
                nc.sync.dma_start(dbg[0:1, 1:2], ch_all[0:1, :])
                nc.sync.dma_start(dbg[0:1, 2:3], t_lo[0:1, :])
                nc.sync.dma_start(dbg[0:1, 3:4], t_hi[0:1, :])

        # ======================= phase 4: dec2 =======================
        mid2P = dram.tile([NC, D_MID, TSL], f32)
        with tc.tile_pool(name="wd2p", bufs=1) as wd2p, \
             tc.tile_pool(name="drh", bufs=3) as drh, \
             tc.tile_pool(name="dst", bufs=4) as dst, \
             tc.tile_pool(name="dps", bufs=6, space="PSUM") as dps:

            wd2_sb = wd2p.tile([128, 24, D_MID], f32)
            nc.sync.dma_start(
                wd2_sb[:], Wd2.rearrange("(ko ki) m -> ki ko m", ki=128))

            for g in range(2):
                for n in range(4):
                    ps6 = [dps.tile([128, 512], f32, tag="dps", name=f"dps{_m}")
                           for _m in range(6)]
                    for kc in range(24):
                        rt = drh.tile([128, 512], f32, tag="rt")
                        nc.sync.dma_start(
                            rt[:], pre1T[kc * 128:(kc + 1) * 128,
                                         n * 512:(n + 1) * 512])
                        mk = drh.tile([128, 512], f32, tag="mk")
                        nc.vector.scalar_tensor_tensor(
                            mk[:], rt[:], t_star[:], rt[:],
                            op0=OP.is_ge, op1=OP.mult)
                        for m6 in range(6):
                            m = g * 6 + m6
                            nc.tensor.matmul(
                                ps6[m6][:],
                                wd2_sb[:, kc, m * 128:(m + 1) * 128],
                                mk[:], start=(kc == 0), stop=(kc == 23))
                    for m6 in range(6):
                        m = g * 6 + m6
                        so = dst.tile([128, 512], f32, tag="dso")
                        nc.scalar.copy(so[:], ps6[m6][:])
                        for h in range(2):
                            c_sl = n * 2 + h
                            nc.sync.dma_start(
                                mid2P[c_sl, m * 128:(m + 1) * 128, :],
                                so[:, h * 256:(h + 1) * 256])

        # ================ phase 5: reduce-scatter + dec1 ================
        rs_out = dram.tile([D_MID, TSL], f32)
        nc.gpsimd.collective_compute(
            "ReduceScatter", mybir.AluOpType.add, replica_groups=RG,
            ins=[mid2P[:].rearrange("c m t -> (c m t)")],
            outs=[rs_out[:].rearrange("m t -> (m t)")])

        with tc.tile_pool(name="d1", bufs=1) as d1, \
             tc.tile_pool(name="d1s", bufs=4) as d1s, \
             tc.tile_pool(name="ops", bufs=3, space="PSUM") as ops:

            wd1_sb = d1.tile([128, 12, D_MODEL], f32)
            nc.sync.dma_start(
                wd1_sb[:], Wd1.rearrange("(ko ki) d -> ki ko d", ki=128))
            xs_sb = d1.tile([128, 6, TSL], f32)
            nc.sync.dma_start(xs_sb[:],
                              xTs.rearrange("(ko ki) t -> ki ko t", ki=128))
            rhm = d1.tile([128, 12, TSL], f32)
            for kc in range(12):
                rh = d1s.tile([128, TSL], f32, tag="rh")
                nc.sync.dma_start(rh[:], rs_out[kc * 128:(kc + 1) * 128, :])
                nc.scalar.activation(rhm[:, kc, :], rh[:], AF.Relu,
                                     bias=bd2_sb[:, kc:kc + 1])
            sse_sb = d1.tile([128, 6], f32)
            for m in range(6):
                pr = ops.tile([128, TSL], f32, tag="ops")
                for kc in range(12):
                    nc.tensor.matmul(pr[:],
                                     wd1_sb[:, kc, m * 128:(m + 1) * 128],
                                     rhm[:, kc, :],
                                     start=(kc == 0), stop=(kc == 11))
                ro = d1s.tile([128, TSL], f32, tag="ro")
                nc.scalar.activation(ro[:], pr[:], AF.Identity,
                                     bias=bd1_sb[:, m:m + 1])
                nc.sync.dma_start(reconT[m * 128:(m + 1) * 128, :], ro[:])
                df = d1s.tile([128, TSL], f32, tag="df")
                nc.vector.tensor_sub(df[:], ro[:], xs_sb[:, m, :])
                jk2 = d1s.tile([128, TSL], f32, tag="jk2")
                nc.scalar.activation(jk2[:], df[:], AF.Square,
                                     accum_out=sse_sb[:, m:m + 1])
            nc.sync.dma_start(sse[:], sse_sb[:])


def _shard_inputs(inputs):
    x = np.ascontiguousarray(np.asarray(inputs["x"], np.float32).reshape(T, D_MODEL))
    xT = np.ascontiguousarray(x.T)
    base = {
        "xT": xT,
        "W1": np.ascontiguousarray(np.asarray(inputs["W_enc1"], np.float32)),
        "b1": np.ascontiguousarray(np.asarray(inputs["b_enc1"], np.float32)),
        "bd2": np.ascontiguousarray(np.asarray(inputs["b_dec2"], np.float32)),
        "Wd1": np.ascontiguousarray(np.asarray(inputs["W_dec1"], np.float32)),
        "bd1": np.ascontiguousarray(np.asarray(inputs["b_dec1"], np.float32)),
    }
    W2 = np.asarray(inputs["W_enc2"], np.float32)
    b2 = np.asarray(inputs["b_enc2"], np.float32)
    Wd2 = np.asarray(inputs["W_dec2"], np.float32)
    in_maps = []
    for c in range(NC):
        m = dict(base)
        m["xTs"] = np.ascontiguousarray(xT[:, c * TSL:(c + 1) * TSL])
        m["W2"] = np.ascontiguousarray(W2[:, c * DF:(c + 1) * DF])
        m["b2"] = np.ascontiguousarray(b2[c * DF:(c + 1) * DF])
        m["Wd2"] = np.ascontiguousarray(Wd2[c * DF:(c + 1) * DF, :])
        in_maps.append(m)
    return in_maps


_CACHE = {}


def _build_bass():
    import concourse.mybir as mybir
    import concourse.tile as tile
    from concourse import bacc

    nc = bacc.Bacc("TRN2", target_bir_lowering=False, debug=False,
                   enable_asserts=False, num_devices=NC)
    f32 = mybir.dt.float32
    ins = {
        "xT": nc.dram_tensor("xT", [D_MODEL, T], f32, kind="ExternalInput").ap(),
        "xTs": nc.dram_tensor("xTs", [D_MODEL, TSL], f32, kind="ExternalInput").ap(),
        "W1": nc.dram_tensor("W1", [D_MODEL, D_MID], f32, kind="ExternalInput").ap(),
        "b1": nc.dram_tensor("b1", [D_MID], f32, kind="ExternalInput").ap(),
        "W2": nc.dram_tensor("W2", [D_MID, DF], f32, kind="ExternalInput").ap(),
        "b2": nc.dram_tensor("b2", [DF], f32, kind="ExternalInput").ap(),
        "Wd2": nc.dram_tensor("Wd2", [DF, D_MID], f32, kind="ExternalInput").ap(),
        "bd2": nc.dram_tensor("bd2", [D_MID], f32, kind="ExternalInput").ap(),
        "Wd1": nc.dram_tensor("Wd1", [D_MID, D_MODEL], f32,
                              kind="ExternalInput").ap(),
        "bd1": nc.dram_tensor("bd1", [D_MODEL], f32, kind="ExternalInput").ap(),
    }
    outs = {
        "reconT": nc.dram_tensor("reconT", [D_MODEL, TSL], f32,
                                 kind="ExternalOutput").ap(),
        "sse": nc.dram_tensor("sse", [128, 6], f32, kind="ExternalOutput").ap(),
        "dbg": nc.dram_tensor("dbg", [1, 8], f32, kind="ExternalOutput").ap(),
    }
    with tile.TileContext(nc) as tc:
        build_tile_kernel(tc, ins, outs)
    nc.finalize()
    return nc


def kernel(**inputs):
    from concourse.bass_utils import run_bass_kernel_spmd

    trace = bool(inputs.pop("_trace", False))
    if "nc" not in _CACHE:
        _CACHE["nc"] = _build_bass()
    nc = _CACHE["nc"]
    in_maps = _shard_inputs(inputs)
    res = run_bass_kernel_spmd(nc, in_maps, core_ids=list(range(NC)),
                               trace=trace)
    _CACHE["last_res"] = res
    recon = np.zeros((T, D_MODEL), np.float32)
    sse_tot = 0.0
    for c in range(NC):
        r = res.results[c]
        recon[c * TSL:(c + 1) * TSL, :] = r["reconT"].T
        sse_tot += float(r["sse"].sum())
    l2 = np.float32(np.float32(sse_tot) / np.float32(T * D_MODEL))
    recon = recon.reshape(B, S, D_MODEL)
    return recon, l2, l2
